# revision 14
# baseline (speedup 1.0000x reference)
"""Trainium2 Bass kernel for nn_BaselineMNISTClassifier (vq_codebook).

reference:
    x = samples - 0.5                        # [B, F]
    hv = einsum('bf,df->bd', x, bhv)         # [B, D]
    e = (hv > 0)                             # binary
    ham[b, c] = sum_d |e - centroids[c, d]|  # [B, C]
    return -ham

Identity used on device: with e' = (hv > 0) - 0.5 in {-1/2, +1/2} and
cmod = 1 - 2c in {-1, +1}:  |e - c| = e' * cmod + 1/2, so
    ham[b, c] = sum_d e'[b, d] * cmod[c, d] + D/2
which turns the broadcast Hamming into a second (tiny) matmul over the
same d-tiles.

Sharding: the D axis (10000) splits across 8 cores, 1250 (zero-padded
to 1280) per core. Every core sees the full batch and computes a
partial hamming [C, B]; the partials sum on the host (padded dims
contribute exactly 0: cmod is 0 there).

fp8 version: the encode matmul runs in fp8 e4m3 with
perf_mode=DoubleRow — each PE cell holds TWO fp8 weights, so one
matmul contracts 256 f-rows (2 per partition) while streaming one
column pair per cycle.  F=784 is zero-padded to 1024 = 4 DoubleRow
steps of 256.  Rounding x and w to e4m3 flips ~0.9% of the encode
bits; the resulting max |ham| error is ~40 on a 5200 scale (rel
8e-3), well inside the 2e-2 gate (verified against the jax reference
in fp8-simulated numpy).  The hamming matmul runs plain fp8 (e' and
cmod are exact in e4m3: +-0.5 / {-1,0,+1}), so the second stage is
exact integer arithmetic given the encode bits.

Both operands are host-converted to fp8 and host-interleaved into the
DoubleRow pair layout [K=128, 2, free] (logical f = ft*256 + i*128 +
p); no on-device transposes or converts anywhere.

Perf structure (per core):
  - 52 warmup matmuls on dummy data release the PE HAM clock gate
    while the inputs stream in
  - input tiles are single-assignment; x DMA triggers issue from SP,
    w/centroid/output triggers from GpSimd
  - fi-outer / bb-inner matmul order: 4 consecutive matmuls share the
    stationary weights, hiding the DoubleRow LDWEIGHTS
  - all four hamming accumulators of a b-group live in ONE PSUM bank
    at partition offsets 0/32/64/96 via col-tiled matmuls
    (tile_position), which frees 7 PSUM banks for the encode
    accumulation
  - hamming matmuls are emitted one d-tile late so the PE never waits
    on the DVE binarize; the epilogue alternates Scalar/DVE and each
    output block DMAs out as soon as its accumulation closes

Toolchain notes: built on bacc.Bacc (its compile() legalizes the
1-sync-wait-per-instruction hardware limit via event semaphores);
output DMAs go through nc.gpsimd because SP DMA_DIRECT2D triggers
only take a single wait.
"""

import sys

sys.path.insert(0, "/opt/trn_rl_repo")

import numpy as np

import concourse.bacc as bacc
import concourse.bass as bass
import concourse.mybir as mybir
import concourse.tile as tile
from concourse.bass_utils import run_bass_kernel_spmd

B = 4096
F = 784
D = 10000
C = 10
CP = 16                      # classes padded so the DoubleRow hamming
                             # LDWEIGHTS pair step is 16B-aligned
NCORES = 8
DREAL = D // NCORES          # 1250 real dims per core
DP = 1280                    # padded to 10 d-tiles of 128
ND = DP // 128               # 10
NB = B // 512                # 8 b-blocks of 512
FPAD = 1024                  # F zero-padded for 4 DoubleRow steps of 256
NF = FPAD // 256             # 4 f-steps (each contracts 2*128 rows)
NWARM = 9                    # PE warmup matmuls (the early encode
                             # matmuls continue the clock ramp)

F32 = mybir.dt.float32
F8 = mybir.dt.float8e4
BF16 = mybir.dt.bfloat16
OP = mybir.AluOpType
AF = mybir.ActivationFunctionType
DR = mybir.MatmulPerfMode.DoubleRow
F8NP = mybir.dt.np(F8)       # ml_dtypes.float8_e4m3

_NC_CACHE = {}


def _build_nc():
    if "nc" in _NC_CACHE:
        return _NC_CACHE["nc"]
    nc = bacc.Bacc("TRN2", debug=False, target_bir_lowering=False)
    # xT rows: ft*128 + p, cols: i*B + b  (logical f = ft*256 + i*128 + p)
    xT = nc.dram_tensor("xT", [FPAD // 2, 2 * B], F8, kind="ExternalInput")
    # wT rows: ft*128 + p, cols: dt*256 + i*128 + dc
    wT = nc.dram_tensor("wT", [FPAD // 2, ND * 256], F8, kind="ExternalInput")
    # cm: DoubleRow pair layout [p, v*ND*CP + pr*2*CP + i*CP + c],
    # CP=16 padded classes (dual-fp8 LDWEIGHTS pair step must be
    # 16B-aligned); variant v=0 holds cmod (for DVE ets in {+-0.5}),
    # v=1 holds cmod/2 (for Scalar Sign ets in {-1,0,+1})
    cm = nc.dram_tensor("cm", [128, 2 * ND * CP], F8, kind="ExternalInput")
    out = nc.dram_tensor("out", [C, B], F32, kind="ExternalOutput")

    with tile.TileContext(nc) as tc:
        with (
            tc.tile_pool(name="dum", bufs=2) as dumpool,
            tc.tile_pool(name="xp", bufs=NB // 2 * NF) as xpool,
            tc.tile_pool(name="wp", bufs=ND * NF) as wpool,
            tc.tile_pool(name="cmp", bufs=1) as cmpool,
            tc.tile_pool(name="ep", bufs=8) as epool,
            tc.tile_pool(name="op", bufs=4) as opool,
            tc.tile_pool(name="pse", bufs=4, space="PSUM") as psepool,
            tc.tile_pool(name="ps2", bufs=4, space="PSUM") as ps2pool,
        ):
            # --- PE warmup: release the HAM clock gate while inputs load.
            # Runs in fp8 DoubleRow so no PE mode switch before the body.
            wdum = dumpool.tile([128, 2, 128], F8)
            nc.gpsimd.memset(wdum[:], 1.0)
            xdum = dumpool.tile([128, 2, 512], F8)
            nc.vector.memset(xdum[:], 1.0)
            psdum = psepool.tile([128, 512], F32, name="psdum", tag="pse")
            for i in range(NWARM):
                nc.tensor.matmul(psdum[:], wdum[:], xdum[:],
                                 start=(i == 0), stop=(i == NWARM - 1),
                                 perf_mode=DR)

            # --- centroid mod weights: one DMA, already fp8 on host, in
            # DoubleRow pair layout: col pr*20 + i*10 + c holds
            # cmod[d=(2pr+i)*128+p, c].
            cmod = cmpool.tile([128, 2 * ND * CP], F8)
            nc.gpsimd.dma_start(cmod[:], cm[:, :])
            cmods = [[cmod[:, v * ND * CP + pr * 2 * CP:
                           v * ND * CP + (pr + 1) * 2 * CP].rearrange(
                "p (i c) -> p i c", i=2) for pr in range(ND // 2)]
                for v in range(2)]

            # --- input loads; tiles single-assignment (loaded once, no
            # slot reuse) so input DMAs never carry data waits. x tiles
            # span two b-blocks in the DoubleRow pair layout [128,2,1024].
            xts = {}
            wts = {}

            def load_x(bp, fi):   # bp = b-block pair index (0..3)
                xt = xpool.tile([128, 2, 1024], F8, name=f"xt_{bp}_{fi}",
                                tag="xt")
                src = xT[fi * 128:(fi + 1) * 128, :].rearrange(
                    "p (i b) -> p i b", i=2)
                nc.sync.dma_start(
                    xt[:], src[:, :, bp * 1024:(bp + 1) * 1024])
                xts[bp, fi] = xt

            def load_w(di, fi):   # one [128, 2, 128] fp8 tile per (d, f)
                wt = wpool.tile([128, 2, 128], F8, name=f"wt_{di}_{fi}",
                                tag="wt")
                nc.gpsimd.dma_start(
                    wt[:],
                    wT[fi * 128:(fi + 1) * 128,
                       di * 256:(di + 1) * 256].rearrange(
                           "p (i dc) -> p i dc", i=2))
                wts[di, fi] = wt

            # bp-major x / d-major w order: the first b-group's first
            # d-tiles land first so the encode can start right after the
            # short warmup
            for bp in (0, 1):
                for fi in range(NF):
                    load_x(bp, fi)
            for bp in (2, 3):
                for fi in range(NF):
                    load_x(bp, fi)
            for di in range(ND):
                for fi in range(NF):
                    load_w(di, fi)

            def xop(bb, fi):
                return xts[bb // 2, fi][:, :, (bb % 2) * 512:(bb % 2 + 1) * 512]

            def wop(di, fi):
                return wts[di, fi][:]

            # --- main compute: two b-groups of 4 blocks.  The hamming
            # runs in DoubleRow too: consecutive d-tile pairs binarize
            # into one [128, 2, 512] fp8 tile, one hamming matmul per
            # pair contracts 256 d-rows.  DoubleRow matmuls must write
            # psum partition 0, so each block owns a PSUM bank (4 ham
            # banks + 4 encode banks); binarize alternates DVE
            # (is_gt-0.5 -> {+-0.5}, cmod variant 0) and Scalar
            # (Sign -> {-1,+1}, cmod/2 variant 1) so the 4-deep encode
            # pool never waits on a single engine.
            NP2 = ND // 2
            for bg in range(2):
                bbs = list(range(bg * 4, bg * 4 + 4))
                psum2 = {}
                for bb in bbs:
                    ps2 = ps2pool.tile([128, 512], F32,
                                       name=f"ps2_{bb % 4}", tag="ps2")
                    psum2[bb] = ps2
                pending = []
                etp = {}
                for di in range(ND):
                    pr = di // 2
                    if di % 2 == 0:
                        for bb in bbs:
                            etp[bb] = epool.tile([128, 1024], F8,
                                                 name=f"et_{pr % 2}_{bb}",
                                                 tag="et")
                    pses = {}
                    for bb in bbs:
                        pses[bb] = psepool.tile([128, 512], F32,
                                                name=f"pse_{di % 2}_{bb}",
                                                tag="pse")
                    for fi in range(NF):
                        for bb in bbs:
                            nc.tensor.matmul(pses[bb][:], wop(di, fi),
                                             xop(bb, fi),
                                             start=(fi == 0),
                                             stop=(fi == NF - 1),
                                             perf_mode=DR)
                    for bb in bbs:
                        # the last d-tile binarizes in halves so the
                        # hamming flush overlaps the binarize
                        dst = etp[bb][:, (di % 2) * 512:(di % 2 + 1) * 512]
                        hs = [slice(h * 256, (h + 1) * 256)
                              for h in range(2)] if di == ND - 1 else [
                                  slice(0, 512)]
                        for sl in hs:
                            if bb % 4 < 2:
                                nc.vector.tensor_scalar(
                                    dst[:, sl], pses[bb][:, sl], 0.0, 0.5,
                                    op0=OP.is_gt, op1=OP.subtract)
                            else:
                                nc.scalar.activation(
                                    dst[:, sl], pses[bb][:, sl], AF.Sign)
                    if di % 2 == 1:
                        for ppr, pbb, pet in pending:
                            nc.tensor.matmul(
                                psum2[pbb][0:CP, :],
                                cmods[(pbb % 4) // 2][ppr],
                                pet[:].rearrange("p (i n) -> p i n", i=2),
                                start=(ppr == 0), stop=(ppr == NP2 - 1),
                                perf_mode=DR)
                        pending = [(pr, bb, etp[bb]) for bb in bbs]
                for ppr, pbb, pet in pending:
                    prhs = pet[:].rearrange("p (i n) -> p i n", i=2)
                    for h in range(2):
                        sl = slice(h * 256, (h + 1) * 256)
                        nc.tensor.matmul(psum2[pbb][0:CP, sl],
                                         cmods[(pbb % 4) // 2][ppr],
                                         prhs[:, :, sl],
                                         start=(ppr == 0),
                                         stop=(ppr == NP2 - 1),
                                         perf_mode=DR)
                    # out = -(psum2 + DREAL/2); alternate engines so the
                    # four epilogues drain in parallel
                    ot = opool.tile([C, 512], F32, name=f"ot_{pbb % 4}",
                                    tag="ot")
                    if pbb % 2 == 0:
                        nc.scalar.activation(ot[:], psum2[pbb][0:C, :],
                                             AF.Copy,
                                             bias=-float(DREAL) / 2.0,
                                             scale=-1.0)
                    else:
                        nc.vector.tensor_scalar(ot[:], psum2[pbb][0:C, :],
                                                float(DREAL) / 2.0, -1.0,
                                                op0=OP.add, op1=OP.mult)
                    eng = nc.scalar if pbb % 2 == 0 else nc.sync
                    eng.dma_start(
                        out[:, pbb * 512:(pbb + 1) * 512], ot[:])
    nc.compile()
    _NC_CACHE["nc"] = nc
    return nc


def _prep_in_maps(samples, bhv_matrix, centroids):
    samples = np.ascontiguousarray(samples, dtype=np.float32)
    bhv_matrix = np.ascontiguousarray(bhv_matrix, dtype=np.float32)
    centroids = np.ascontiguousarray(centroids, dtype=np.float32)

    # x: [B, F] -> fp8 pair layout [FPAD/2, 2*B]; row ft*128+p, col i*B+b
    # holds x'[b, f=ft*256+i*128+p].
    x8 = np.zeros((FPAD, B), dtype=F8NP)
    x8[:F, :] = (samples.T - np.float32(0.5)).astype(F8NP)
    x8 = np.ascontiguousarray(
        x8.reshape(NF, 2, 128, B).transpose(0, 2, 1, 3).reshape(
            FPAD // 2, 2 * B))

    in_maps = []
    for k in range(NCORES):
        lo_, hi_ = k * DREAL, (k + 1) * DREAL
        # w: fp8 pair layout [FPAD/2, ND*256]; row ft*128+p,
        # col dt*256+i*128+dc holds bhv[lo+dt*128+dc, f=ft*256+i*128+p].
        wk = np.zeros((FPAD, DP), dtype=F8NP)
        wk[:F, :DREAL] = bhv_matrix[lo_:hi_, :].T.astype(F8NP)
        wk = np.ascontiguousarray(
            wk.reshape(NF, 2, 128, ND, 128).transpose(0, 2, 3, 1, 4).reshape(
                FPAD // 2, ND * 256))
        # cmod: [128, 2*ND*CP] fp8 in DoubleRow pair layout (col
        # v*ND*CP + pr*2*CP + i*CP + c = scale_v * cmod[(2pr+i)*128+p, c]
        # with scale_0 = 1, scale_1 = 0.5); 1-2c real, 0 pads
        cmk = np.zeros((DP, CP), dtype=np.float32)
        cmk[:DREAL, :C] = 1.0 - 2.0 * centroids[:, lo_:hi_].T
        cmk = cmk.reshape(ND // 2, 2, 128, CP).transpose(2, 0, 1, 3).reshape(
            128, ND * CP)
        cmk = np.ascontiguousarray(
            np.concatenate([cmk, 0.5 * cmk], axis=1)).astype(F8NP)
        in_maps.append({"xT": x8, "wT": wk, "cm": cmk})
    return in_maps


def _run(samples, bhv_matrix, centroids, **spmd_kwargs):
    nc = _build_nc()
    in_maps = _prep_in_maps(samples, bhv_matrix, centroids)
    res = run_bass_kernel_spmd(nc, in_maps, core_ids=list(range(NCORES)),
                               **spmd_kwargs)
    acc = np.zeros((C, B), dtype=np.float32)
    for r in res.results:
        acc += r["out"]
    return np.ascontiguousarray(acc.T), res


def kernel(samples, bhv_matrix, centroids):
    out, _ = _run(samples, bhv_matrix, centroids)
    return out
